# revision 25
# baseline (speedup 1.0000x reference)
"""Trainium2 Bass kernel for nn_LocalExperts (MoE grouped FFN).

out[e] = relu(x[e] @ wi[e]) @ wo[e]   for e in 0..7

Expert-parallel over 8 NeuronCores: core e computes expert e's FFN.
Per-core work: x [8192, 512] f32, wi [512, 2048], wo [2048, 512]
  GEMM1: hT[f, m] = wi[d, f].T @ xT[d, m]  (accumulate over 4 d-chunks)
  relu (ScalarE) -> hT in SBUF as bf16
  GEMM2: out[m, d] = hT[f, m].T @ wo[f, d] (accumulate over 16 f-chunks)

v7 = the gapless v2 schedule, all matmuls in bf16:
- bf16 halves LDWEIGHTS time, dropping per-matmul stream cost from
  227.1 to 215.8ns (512 cols); bf16 PE transposes run 1 cycle/row vs
  1.5 for f32r. End-to-end rel err ~2.8e-3 (gate 2e-2).
- x fp32 -> SBUF DMA, one wide DVE cast -> bf16, then PE-transposed
  (bf16 identity) into xT. The transpose of m-tile t+1 runs between
  GEMM1(t) and GEMM2(t): it fills exactly the PE bubble where GEMM2
  would otherwise wait for the last relu of hT(t). (DMA XBAR transpose
  variants were tried and lose: each costs ~1.3us of hwdge queue issue
  time and its semaphore waits stall whichever engine queue hosts it —
  Tile deps are per-engine completion counters, so a waiting
  instruction blocks everything behind it on that queue.)
- Weights DMA'd fp32 in chunks (after x tile 0 on the SP queue), DVE
  cast to bf16; GEMM groups only depend on the chunk they read, so the
  PE starts ~15us in.
"""

import numpy as np

import concourse.mybir as mybir
from concourse import bacc
from concourse.tile import TileContext
from concourse.bass_utils import run_bass_kernel_spmd
from concourse.masks import make_identity

E, W, C, D, F = 8, 8, 1024, 512, 2048
P = 128
M_TOT = W * C            # 8192 rows per expert
M_TILE = 512             # rows per m-tile
N_MT = M_TOT // M_TILE   # 16
MS = M_TILE // P         # 4 m-subtiles of 128 rows
DC = D // P              # 4 d-chunks
FC = F // P              # 16 f-chunks

F32 = mybir.dt.float32
BF16 = mybir.dt.bfloat16


def _build_nc():
    nc = bacc.Bacc(None, target_bir_lowering=False)

    x = nc.dram_tensor("x", [M_TOT, D], F32, kind="ExternalInput")
    wi = nc.dram_tensor("wi", [D, F], F32, kind="ExternalInput")
    wo = nc.dram_tensor("wo", [F, D], F32, kind="ExternalInput")
    out = nc.dram_tensor("out", [M_TOT, D], F32, kind="ExternalOutput")

    x_v = x.rearrange("(mt ms p) d -> mt p ms d", p=P, ms=MS)
    out_v = out.rearrange("(mt ms p) d -> mt p ms d", p=P, ms=MS)
    wi_v = wi.rearrange("(dc p) f -> p dc f", p=P)
    wo_v = wo.rearrange("(fc p) d -> p fc d", p=P)

    with TileContext(nc) as tc:
        with (
            tc.tile_pool(name="const", bufs=1) as cpool,
            tc.tile_pool(name="xin", bufs=3) as xin_pool,
            tc.tile_pool(name="xbf", bufs=2) as xbf_pool,
            tc.tile_pool(name="xt", bufs=2) as xt_pool,
            tc.tile_pool(name="ht", bufs=2) as ht_pool,
            tc.tile_pool(name="stg", bufs=4) as stg_pool,
            tc.tile_pool(name="osb", bufs=4) as o_pool,
            tc.tile_pool(name="tp_ps", bufs=2, space="PSUM") as tp_psum,
            tc.tile_pool(name="h_ps", bufs=2, space="PSUM") as h_psum,
            tc.tile_pool(name="o_ps", bufs=2, space="PSUM") as o_psum,
        ):
            ident = cpool.tile([P, P], BF16)
            ident_f32 = cpool.tile([P, P], F32)
            make_identity(nc, ident_f32)
            nc.vector.tensor_copy(ident, ident_f32)

            wi_bf = cpool.tile([P, DC, F], BF16)
            wo_bf = cpool.tile([P, FC, D], BF16)

            def load_x(mt):
                x_nat = xin_pool.tile([P, MS, D], F32)
                nc.sync.dma_start(x_nat, x_v[mt])
                x_bf = xbf_pool.tile([P, MS, D], BF16)
                nc.vector.tensor_copy(x_bf, x_nat)
                return x_bf

            def load_wi(q):
                s = slice(q * (F // 8), (q + 1) * (F // 8))
                st = stg_pool.tile([P, DC, F // 8], F32, tag="stg")
                nc.sync.dma_start(st, wi_v[:, :, s])
                nc.vector.tensor_copy(wi_bf[:, :, s], st)

            # x tile 0 queued before any weight bytes so the transpose
            # pipeline starts immediately.
            x_bf0 = load_x(0)

            # Weights in 512KB chunks, ordered so every transfer lands
            # just before its consumer: wi cols 0-1023 (GEMM1(0) reads
            # them from ~15us), then x tile 1 (transpose(1) at ~29us),
            # then wi cols 1024-2047, then wo (GEMM2(0) consumes its
            # chunks 30-43us in, right as they arrive).
            for q in range(4):
                load_wi(q)
            x_bf1 = load_x(1)
            for q in range(4, 8):
                load_wi(q)
            for q in range(8):
                s = slice(q * (FC // 8), (q + 1) * (FC // 8))
                st = stg_pool.tile([P, FC // 8, D], F32, tag="stg")
                nc.sync.dma_start(st, wo_v[:, s])
                nc.vector.tensor_copy(wo_bf[:, s], st)

            def transpose_x(x_bf):
                # xT [d, m]: per m-subtile, 4 PE transposes (bf16, 1
                # cycle/row) form ONE psum accumulation group in ONE bank
                # (start only on the first, disjoint 128-col regions),
                # drained by ONE wide DVE copy.
                xt = xt_pool.tile([P, DC, M_TILE], BF16)
                for ms in range(MS):
                    tp = tp_psum.tile([P, DC, P], BF16)
                    for dc in range(DC):
                        nc.tensor.matmul(
                            tp[:, dc],
                            x_bf[:, ms, dc * P : (dc + 1) * P],
                            ident,
                            is_transpose=True,
                            start=(dc == 0),
                            stop=(dc == DC - 1),
                            skip_group_check=True,
                        )
                    nc.vector.tensor_copy(xt[:, :, ms * P : (ms + 1) * P], tp)
                return xt

            def gemm1(xt):
                # hT[f, m]; two 4-matmul PSUM groups (adjacent banks of one
                # 2-bank tile) drained by a single ACT relu -> bf16 SBUF.
                hT = ht_pool.tile([P, FC, M_TILE], BF16)
                for fc2 in range(FC // 2):
                    hp = h_psum.tile([P, 2, M_TILE], F32)
                    for half in range(2):
                        fc = 2 * fc2 + half
                        for dc in range(DC):
                            nc.tensor.matmul(
                                hp[:, half],
                                wi_bf[:, dc, fc * P : (fc + 1) * P],
                                xt[:, dc, :],
                                start=(dc == 0),
                                stop=(dc == DC - 1),
                            )
                    nc.scalar.activation(
                        hT[:, 2 * fc2 : 2 * fc2 + 2, :],
                        hp,
                        mybir.ActivationFunctionType.Relu,
                    )
                return hT

            def gemm2(mt, hT):
                # out[m, d] per 128-row subtile
                for ms in range(MS):
                    op = o_psum.tile([P, D], F32)
                    for fc in range(FC):
                        nc.tensor.matmul(
                            op,
                            hT[:, fc, ms * P : (ms + 1) * P],
                            wo_bf[:, fc, :],
                            start=(fc == 0),
                            stop=(fc == FC - 1),
                        )
                    o_t = o_pool.tile([P, D], F32)
                    nc.vector.tensor_copy(o_t, op)
                    nc.sync.dma_start(out_v[mt, :, ms, :], o_t)

            # software pipeline: transpose m-tile t+1 between G1(t) and
            # G2(t) so the xt drain latency hides under GEMM2's matmuls.
            xt = transpose_x(x_bf0)
            for mt in range(N_MT):
                hT = gemm1(xt)
                if mt + 1 < N_MT:
                    xt = transpose_x(x_bf1 if mt == 0 else load_x(mt + 1))
                gemm2(mt, hT)

    nc.finalize()
    return nc


_CACHE = {}


def _get_nc():
    if "nc" not in _CACHE:
        _CACHE["nc"] = _build_nc()
    return _CACHE["nc"]


def _run(x, wi, wo, **spmd_kwargs):
    """x [E, 8192, 512], wi [E, 512, 2048], wo [E, 2048, 512] -> results."""
    nc = _get_nc()
    in_maps = [
        {
            "x": np.ascontiguousarray(x[e]),
            "wi": np.ascontiguousarray(wi[e]),
            "wo": np.ascontiguousarray(wo[e]),
        }
        for e in range(E)
    ]
    return nc, run_bass_kernel_spmd(nc, in_maps, core_ids=list(range(E)), **spmd_kwargs)


def kernel(dispatched_hidden_states, experts_capacity_usage=None, wi=None, wo=None):
    x = np.asarray(dispatched_hidden_states, dtype=np.float32).reshape(E, M_TOT, D)
    wi_ = np.asarray(wi, dtype=np.float32)
    wo_ = np.asarray(wo, dtype=np.float32)
    _, res = _run(x, wi_, wo_)
    out = np.stack([res.results[e]["out"] for e in range(E)])
    return out.reshape(E, W, C, D)


# revision 26
# speedup vs baseline: 1.1733x; 1.1733x over previous
"""Trainium2 Bass kernel for nn_LocalExperts (MoE grouped FFN).

out[e] = relu(x[e] @ wi[e]) @ wo[e]   for e in 0..7

Expert-parallel over 8 NeuronCores: core e computes expert e's FFN.
Per-core work: x [8192, 512] f32, wi [512, 2048], wo [2048, 512]
  GEMM1: hT[f, m] = wi[d, f].T @ xT[d, m]  (accumulate over 4 d-chunks)
  relu (ScalarE) -> hT in SBUF as bf16
  GEMM2: out[m, d] = hT[f, m].T @ wo[f, d] (accumulate over 16 f-chunks)

v7 = the gapless v2 schedule, all matmuls in bf16:
- bf16 halves LDWEIGHTS time, dropping per-matmul stream cost from
  227.1 to 215.8ns (512 cols); bf16 PE transposes run 1 cycle/row vs
  1.5 for f32r. End-to-end rel err ~2.8e-3 (gate 2e-2).
- x fp32 -> SBUF DMA, one wide DVE cast -> bf16, then PE-transposed
  (bf16 identity) into xT. The transpose of m-tile t+1 runs between
  GEMM1(t) and GEMM2(t): it fills exactly the PE bubble where GEMM2
  would otherwise wait for the last relu of hT(t). (DMA XBAR transpose
  variants were tried and lose: each costs ~1.3us of hwdge queue issue
  time and its semaphore waits stall whichever engine queue hosts it —
  Tile deps are per-engine completion counters, so a waiting
  instruction blocks everything behind it on that queue.)
- Weights DMA'd fp32 in chunks (after x tile 0 on the SP queue), DVE
  cast to bf16; GEMM groups only depend on the chunk they read, so the
  PE starts ~15us in.
"""

import numpy as np

import concourse.mybir as mybir
from concourse import bacc
from concourse.tile import TileContext
from concourse.bass_utils import run_bass_kernel_spmd
from concourse.masks import make_identity

E, W, C, D, F = 8, 8, 1024, 512, 2048
P = 128
M_TOT = W * C            # 8192 rows per expert
M_TILE = 512             # rows per m-tile
N_MT = M_TOT // M_TILE   # 16
MS = M_TILE // P         # 4 m-subtiles of 128 rows
DC = D // P              # 4 d-chunks
FC = F // P              # 16 f-chunks

F32 = mybir.dt.float32
BF16 = mybir.dt.bfloat16


def _build_nc():
    nc = bacc.Bacc(None, target_bir_lowering=False)

    x = nc.dram_tensor("x", [M_TOT, D], F32, kind="ExternalInput")
    wi = nc.dram_tensor("wi", [D, F], F32, kind="ExternalInput")
    wo = nc.dram_tensor("wo", [F, D], F32, kind="ExternalInput")
    out = nc.dram_tensor("out", [M_TOT, D], F32, kind="ExternalOutput")

    x_v = x.rearrange("(mt ms p) d -> mt p ms d", p=P, ms=MS)
    out_v = out.rearrange("(mt ms p) d -> mt p ms d", p=P, ms=MS)
    wi_v = wi.rearrange("(dc p) f -> p dc f", p=P)
    wo_v = wo.rearrange("(fc p) d -> p fc d", p=P)

    with TileContext(nc) as tc:
        with (
            tc.tile_pool(name="const", bufs=1) as cpool,
            tc.tile_pool(name="xin", bufs=3) as xin_pool,
            tc.tile_pool(name="xbf", bufs=2) as xbf_pool,
            tc.tile_pool(name="xt", bufs=2) as xt_pool,
            tc.tile_pool(name="ht", bufs=2) as ht_pool,
            tc.tile_pool(name="stg", bufs=2) as stg_pool,
            tc.tile_pool(name="osb", bufs=4) as o_pool,
            tc.tile_pool(name="tp_ps", bufs=2, space="PSUM") as tp_psum,
            tc.tile_pool(name="h_ps", bufs=2, space="PSUM") as h_psum,
            tc.tile_pool(name="o_ps", bufs=2, space="PSUM") as o_psum,
        ):
            ident = cpool.tile([P, P], BF16)
            ident_f32 = cpool.tile([P, P], F32)
            make_identity(nc, ident_f32)
            nc.vector.tensor_copy(ident, ident_f32)

            wi_bf = cpool.tile([P, DC, F], BF16)
            wo_bf = cpool.tile([P, FC, D], BF16)

            def load_x(mt):
                x_nat = xin_pool.tile([P, MS, D], F32)
                nc.sync.dma_start(x_nat, x_v[mt])
                x_bf = xbf_pool.tile([P, MS, D], BF16)
                nc.vector.tensor_copy(x_bf, x_nat)
                return x_bf

            # x tile 0 queued before any weight bytes so the transpose
            # pipeline starts immediately.
            x_bf0 = load_x(0)

            # Weights: fp32 DMA chunks -> staging, DVE cast -> bf16,
            # wi first (GEMM1 needs its low chunks ~15us in).
            for q in range(4):
                s = slice(q * (F // 4), (q + 1) * (F // 4))
                st = stg_pool.tile([P, DC, F // 4], F32, tag="stg")
                nc.sync.dma_start(st, wi_v[:, :, s])
                nc.vector.tensor_copy(wi_bf[:, :, s], st)
            for q in range(4):
                s = slice(q * (FC // 4), (q + 1) * (FC // 4))
                st = stg_pool.tile([P, FC // 4, D], F32, tag="stg")
                nc.sync.dma_start(st, wo_v[:, s])
                nc.vector.tensor_copy(wo_bf[:, s], st)

            def transpose_x(x_bf):
                # xT [d, m]: per m-subtile, 4 PE transposes (bf16, 1
                # cycle/row) form ONE psum accumulation group in ONE bank
                # (start only on the first, disjoint 128-col regions),
                # drained by ONE wide DVE copy.
                xt = xt_pool.tile([P, DC, M_TILE], BF16)
                for ms in range(MS):
                    tp = tp_psum.tile([P, DC, P], BF16)
                    for dc in range(DC):
                        nc.tensor.matmul(
                            tp[:, dc],
                            x_bf[:, ms, dc * P : (dc + 1) * P],
                            ident,
                            is_transpose=True,
                            start=(dc == 0),
                            stop=(dc == DC - 1),
                            skip_group_check=True,
                        )
                    nc.vector.tensor_copy(xt[:, :, ms * P : (ms + 1) * P], tp)
                return xt

            def gemm1(xt):
                # hT[f, m]; two 4-matmul PSUM groups (adjacent banks of one
                # 2-bank tile) drained by a single ACT relu -> bf16 SBUF.
                hT = ht_pool.tile([P, FC, M_TILE], BF16)
                for fc2 in range(FC // 2):
                    hp = h_psum.tile([P, 2, M_TILE], F32)
                    for half in range(2):
                        fc = 2 * fc2 + half
                        for dc in range(DC):
                            nc.tensor.matmul(
                                hp[:, half],
                                wi_bf[:, dc, fc * P : (fc + 1) * P],
                                xt[:, dc, :],
                                start=(dc == 0),
                                stop=(dc == DC - 1),
                            )
                    nc.scalar.activation(
                        hT[:, 2 * fc2 : 2 * fc2 + 2, :],
                        hp,
                        mybir.ActivationFunctionType.Relu,
                    )
                return hT

            def gemm2(mt, hT):
                # out[m, d] per 128-row subtile
                for ms in range(MS):
                    op = o_psum.tile([P, D], F32)
                    for fc in range(FC):
                        nc.tensor.matmul(
                            op,
                            hT[:, fc, ms * P : (ms + 1) * P],
                            wo_bf[:, fc, :],
                            start=(fc == 0),
                            stop=(fc == FC - 1),
                        )
                    o_t = o_pool.tile([P, D], F32)
                    nc.vector.tensor_copy(o_t, op)
                    nc.sync.dma_start(out_v[mt, :, ms, :], o_t)

            # software pipeline: transpose m-tile t+1 between G1(t) and
            # G2(t) so the xt drain latency hides under GEMM2's matmuls.
            xt = transpose_x(x_bf0)
            for mt in range(N_MT):
                hT = gemm1(xt)
                if mt + 1 < N_MT:
                    xt = transpose_x(load_x(mt + 1))
                gemm2(mt, hT)

    nc.finalize()
    return nc


_CACHE = {}


def _get_nc():
    if "nc" not in _CACHE:
        _CACHE["nc"] = _build_nc()
    return _CACHE["nc"]


def _run(x, wi, wo, **spmd_kwargs):
    """x [E, 8192, 512], wi [E, 512, 2048], wo [E, 2048, 512] -> results."""
    nc = _get_nc()
    in_maps = [
        {
            "x": np.ascontiguousarray(x[e]),
            "wi": np.ascontiguousarray(wi[e]),
            "wo": np.ascontiguousarray(wo[e]),
        }
        for e in range(E)
    ]
    return nc, run_bass_kernel_spmd(nc, in_maps, core_ids=list(range(E)), **spmd_kwargs)


def kernel(dispatched_hidden_states, experts_capacity_usage=None, wi=None, wo=None):
    x = np.asarray(dispatched_hidden_states, dtype=np.float32).reshape(E, M_TOT, D)
    wi_ = np.asarray(wi, dtype=np.float32)
    wo_ = np.asarray(wo, dtype=np.float32)
    _, res = _run(x, wi_, wo_)
    out = np.stack([res.results[e]["out"] for e in range(E)])
    return out.reshape(E, W, C, D)


# revision 28
# speedup vs baseline: 1.1932x; 1.0169x over previous
"""Trainium2 Bass kernel for nn_LocalExperts (MoE grouped FFN).

out[e] = relu(x[e] @ wi[e]) @ wo[e]   for e in 0..7

Expert-parallel over 8 NeuronCores: core e computes expert e's FFN.
Per-core work: x [8192, 512] f32, wi [512, 2048], wo [2048, 512]
  GEMM1: hT[f, m] = wi[d, f].T @ xT[d, m]  (accumulate over 4 d-chunks)
  relu (ScalarE) -> hT in SBUF as bf16
  GEMM2: out[m, d] = hT[f, m].T @ wo[f, d] (accumulate over 16 f-chunks)

v7 = the gapless v2 schedule, all matmuls in bf16:
- bf16 halves LDWEIGHTS time, dropping per-matmul stream cost from
  227.1 to 215.8ns (512 cols); bf16 PE transposes run 1 cycle/row vs
  1.5 for f32r. End-to-end rel err ~2.8e-3 (gate 2e-2).
- x fp32 -> SBUF DMA, one wide DVE cast -> bf16, then PE-transposed
  (bf16 identity) into xT. The transpose of m-tile t+1 runs between
  GEMM1(t) and GEMM2(t): it fills exactly the PE bubble where GEMM2
  would otherwise wait for the last relu of hT(t). (DMA XBAR transpose
  variants were tried and lose: each costs ~1.3us of hwdge queue issue
  time and its semaphore waits stall whichever engine queue hosts it —
  Tile deps are per-engine completion counters, so a waiting
  instruction blocks everything behind it on that queue.)
- Weights DMA'd fp32 in chunks (after x tile 0 on the SP queue), DVE
  cast to bf16; GEMM groups only depend on the chunk they read, so the
  PE starts ~15us in.
"""

import numpy as np

import concourse.mybir as mybir
from concourse import bacc
from concourse.tile import TileContext
from concourse.bass_utils import run_bass_kernel_spmd
from concourse.masks import make_identity

E, W, C, D, F = 8, 8, 1024, 512, 2048
P = 128
M_TOT = W * C            # 8192 rows per expert
M_TILE = 512             # rows per m-tile
N_MT = M_TOT // M_TILE   # 16
MS = M_TILE // P         # 4 m-subtiles of 128 rows
DC = D // P              # 4 d-chunks
FC = F // P              # 16 f-chunks

F32 = mybir.dt.float32
BF16 = mybir.dt.bfloat16


def _build_nc():
    nc = bacc.Bacc(None, target_bir_lowering=False)

    x = nc.dram_tensor("x", [M_TOT, D], F32, kind="ExternalInput")
    wi = nc.dram_tensor("wi", [D, F], F32, kind="ExternalInput")
    wo = nc.dram_tensor("wo", [F, D], F32, kind="ExternalInput")
    out = nc.dram_tensor("out", [M_TOT, D], F32, kind="ExternalOutput")

    x_v = x.rearrange("(mt ms p) d -> mt p ms d", p=P, ms=MS)
    out_v = out.rearrange("(mt ms p) d -> mt p ms d", p=P, ms=MS)
    wi_v = wi.rearrange("(dc p) f -> p dc f", p=P)
    wo_v = wo.rearrange("(fc p) d -> p fc d", p=P)

    with TileContext(nc) as tc:
        with (
            tc.tile_pool(name="const", bufs=1) as cpool,
            tc.tile_pool(name="xin", bufs=3) as xin_pool,
            tc.tile_pool(name="xbf", bufs=2) as xbf_pool,
            tc.tile_pool(name="xt", bufs=2) as xt_pool,
            tc.tile_pool(name="ht", bufs=2) as ht_pool,
            tc.tile_pool(name="stg", bufs=2) as stg_pool,
            tc.tile_pool(name="osb", bufs=4) as o_pool,
            tc.tile_pool(name="tp_ps", bufs=2, space="PSUM") as tp_psum,
            tc.tile_pool(name="h_ps", bufs=2, space="PSUM") as h_psum,
            tc.tile_pool(name="o_ps", bufs=2, space="PSUM") as o_psum,
        ):
            ident = cpool.tile([P, P], BF16)
            ident_f32 = cpool.tile([P, P], F32)
            make_identity(nc, ident_f32)
            nc.vector.tensor_copy(ident, ident_f32)

            wi_bf = cpool.tile([P, DC, F], BF16)
            wo_bf = cpool.tile([P, FC, D], BF16)

            def load_x(mt):
                x_nat = xin_pool.tile([P, MS, D], F32)
                nc.sync.dma_start(x_nat, x_v[mt])
                x_bf = xbf_pool.tile([P, MS, D], BF16)
                nc.vector.tensor_copy(x_bf, x_nat)
                return x_bf

            # x tile 0 queued before any weight bytes so the transpose
            # pipeline starts immediately.
            x_bf0 = load_x(0)

            # Weights: fp32 DMA chunks -> staging, DVE cast -> bf16,
            # wi first (GEMM1 needs its low chunks ~15us in).
            for q in range(4):
                s = slice(q * (F // 4), (q + 1) * (F // 4))
                st = stg_pool.tile([P, DC, F // 4], F32, tag="stg")
                nc.sync.dma_start(st, wi_v[:, :, s])
                nc.vector.tensor_copy(wi_bf[:, :, s], st)
            # x tile 1 ahead of wo: transpose(1) needs it at ~29us, while
            # GEMM2(0) doesn't touch wo until ~30us and tolerates its
            # chunks arriving through ~43us.
            x_bf1 = load_x(1)
            for q in range(4):
                s = slice(q * (FC // 4), (q + 1) * (FC // 4))
                st = stg_pool.tile([P, FC // 4, D], F32, tag="stg")
                nc.sync.dma_start(st, wo_v[:, s])
                nc.vector.tensor_copy(wo_bf[:, s], st)

            def transpose_x(x_bf):
                # xT [d, m]: per m-subtile, 4 PE transposes (bf16, 1
                # cycle/row) form ONE psum accumulation group in ONE bank
                # (start only on the first, disjoint 128-col regions),
                # drained by ONE wide DVE copy.
                xt = xt_pool.tile([P, DC, M_TILE], BF16)
                for ms in range(MS):
                    tp = tp_psum.tile([P, DC, P], BF16)
                    for dc in range(DC):
                        nc.tensor.matmul(
                            tp[:, dc],
                            x_bf[:, ms, dc * P : (dc + 1) * P],
                            ident,
                            is_transpose=True,
                            start=(dc == 0),
                            stop=(dc == DC - 1),
                            skip_group_check=True,
                        )
                    nc.vector.tensor_copy(xt[:, :, ms * P : (ms + 1) * P], tp)
                return xt

            def gemm1(xt):
                # hT[f, m]; two 4-matmul PSUM groups (adjacent banks of one
                # 2-bank tile) drained by a single ACT relu -> bf16 SBUF.
                hT = ht_pool.tile([P, FC, M_TILE], BF16)
                for fc2 in range(FC // 2):
                    hp = h_psum.tile([P, 2, M_TILE], F32)
                    for half in range(2):
                        fc = 2 * fc2 + half
                        for dc in range(DC):
                            nc.tensor.matmul(
                                hp[:, half],
                                wi_bf[:, dc, fc * P : (fc + 1) * P],
                                xt[:, dc, :],
                                start=(dc == 0),
                                stop=(dc == DC - 1),
                            )
                    nc.scalar.activation(
                        hT[:, 2 * fc2 : 2 * fc2 + 2, :],
                        hp,
                        mybir.ActivationFunctionType.Relu,
                    )
                return hT

            def gemm2(mt, hT):
                # out[m, d] per 128-row subtile
                for ms in range(MS):
                    op = o_psum.tile([P, D], F32)
                    for fc in range(FC):
                        nc.tensor.matmul(
                            op,
                            hT[:, fc, ms * P : (ms + 1) * P],
                            wo_bf[:, fc, :],
                            start=(fc == 0),
                            stop=(fc == FC - 1),
                        )
                    o_t = o_pool.tile([P, D], F32)
                    nc.vector.tensor_copy(o_t, op)
                    nc.sync.dma_start(out_v[mt, :, ms, :], o_t)

            # software pipeline: transpose m-tile t+1 between G1(t) and
            # G2(t) so the xt drain latency hides under GEMM2's matmuls.
            xt = transpose_x(x_bf0)
            for mt in range(N_MT):
                hT = gemm1(xt)
                if mt + 1 < N_MT:
                    xt = transpose_x(x_bf1 if mt == 0 else load_x(mt + 1))
                gemm2(mt, hT)

    nc.finalize()
    return nc


_CACHE = {}


def _get_nc():
    if "nc" not in _CACHE:
        _CACHE["nc"] = _build_nc()
    return _CACHE["nc"]


def _run(x, wi, wo, **spmd_kwargs):
    """x [E, 8192, 512], wi [E, 512, 2048], wo [E, 2048, 512] -> results."""
    nc = _get_nc()
    in_maps = [
        {
            "x": np.ascontiguousarray(x[e]),
            "wi": np.ascontiguousarray(wi[e]),
            "wo": np.ascontiguousarray(wo[e]),
        }
        for e in range(E)
    ]
    return nc, run_bass_kernel_spmd(nc, in_maps, core_ids=list(range(E)), **spmd_kwargs)


def kernel(dispatched_hidden_states, experts_capacity_usage=None, wi=None, wo=None):
    x = np.asarray(dispatched_hidden_states, dtype=np.float32).reshape(E, M_TOT, D)
    wi_ = np.asarray(wi, dtype=np.float32)
    wo_ = np.asarray(wo, dtype=np.float32)
    _, res = _run(x, wi_, wo_)
    out = np.stack([res.results[e]["out"] for e in range(E)])
    return out.reshape(E, W, C, D)
